# revision 16
# baseline (speedup 1.0000x reference)
"""Batched cosine-similarity matrix (retrieval_knn) on 8 TRN2 NeuronCores.

reference:  out[b, n, m] = <x[b,n,:], y[b,m,:]> / max(||x[b,n]|| * ||y[b,m]||, 1e-8)
shapes:     x, y: [8, 2048, 512] f32  ->  out: [8, 2048, 2048] f32

Sharding: data-parallel over the batch dim - batch b runs on core b.

v6 (v1 133us, v2 106us, v4 97.6us, v5 93.2us):
  - All-bf16 data path; 4MB input split across both HWDGE queues
    (x on sync, y on ACT); bf16 output upcast on host.
  - v5's limiter was the DVE (busy 67us): every drain is a 750ns
    fp32-PSUM stt (PSUM source pins the DVE at 1x mode). v6 splits the
    epilogue: half the (t,c) groups drain on ACT - Copy(ps * rx) with a
    per-partition scale AP into bf16 - and finish with a 327ns bf16 2x
    DVE multiply by ry; the other half keep the single 750ns DVE stt.
    ~44us on each engine instead of 67 on one.
  - First 8 groups defer: plain DVE copy to SBUF staging (no rx/ry
    dependency), scaled later - the PE streams through the norm tail.
  - ry in bf16 (needed for the 2x DVE multiply; adds ~1e-3 rel err).
  - ACT's single table slot: all Lns then all Exps (2 loads + 1 for
    the hoisted first), after v4 measured 6 alternating reloads.
  - rx via per-k N=1 matmuls start=stop=True (PSUM has_written is
    bank-granular), DVE-accumulated; squares on DVE during the load.
  - Dummy warm-up matmuls bridge chunk waits (HAM stays at K=8/8).
"""

import numpy as np
import ml_dtypes

import concourse.bass as bass
import concourse.bacc as bacc
import concourse.mybir as mybir
import concourse.tile as tile
from concourse.bass_utils import run_bass_kernel_spmd

P = 128          # partitions
D = 512          # feature dim (contraction)
N = 2048         # rows of x / y
B = 8            # batch == n_cores
KC = D // P      # 4 k-chunks
NT = N // P      # 16 n-tiles (output partition tiles)
MC = N // 512    # 4 m-chunks (PSUM-bank width)
WARMUP = 36      # initial dummy matmuls (~3.8us @1.2GHz) to flip HAM to 8/8
FILLS = (20, 10, 8, 0)  # dummies sized to v11-measured PE gaps
DEFER = 8        # groups drained unscaled to SBUF (PE slack over norm tail)

F32 = mybir.dt.float32
BF16 = mybir.dt.bfloat16

_CACHED = {}


def _build_nc(variant: str = "v12") -> bass.Bass:
    """Build the single-core Bass program (same program runs SPMD on 8 cores)."""
    nc = bacc.Bacc(trn_type="TRN2", target_bir_lowering=False, debug=False)

    xT = nc.dram_tensor("xT", [D, N], BF16, kind="ExternalInput").ap()
    yT = nc.dram_tensor("yT", [D, N], BF16, kind="ExternalInput").ap()
    out = nc.dram_tensor("out", [N, N], BF16, kind="ExternalOutput").ap()

    Ln = mybir.ActivationFunctionType.Ln
    Exp = mybir.ActivationFunctionType.Exp
    Copy = mybir.ActivationFunctionType.Copy
    mult = mybir.AluOpType.mult

    with tile.TileContext(nc) as tc:
        with (
            tc.tile_pool(name="consts", bufs=1) as const_pool,
            tc.tile_pool(name="xin", bufs=1) as xin_pool,
            tc.tile_pool(name="yin", bufs=1) as yin_pool,
            tc.tile_pool(name="sq", bufs=1) as sq_pool,
            tc.tile_pool(name="norms", bufs=1) as norm_pool,
            tc.tile_pool(name="defer", bufs=1) as defer_pool,
            tc.tile_pool(name="tmp", bufs=4) as tmp_pool,
            tc.tile_pool(name="ostage", bufs=5) as out_pool,
            tc.tile_pool(name="mm_ps", bufs=3, space="PSUM") as mm_ps_pool,
            tc.tile_pool(name="ry_ps", bufs=1, space="PSUM") as ry_ps_pool,
            tc.tile_pool(name="rx_ps", bufs=1, space="PSUM") as rx_ps_pool,
        ):
            ones = const_pool.tile([P, P], BF16, name="ones")
            nc.vector.memset(ones, 1.0)

            def dummy_mms(n):
                # junk matmuls with no input deps; they run whenever the
                # PE would otherwise idle waiting on a DMA chunk, keeping
                # the HAM activity window busy (no K=4/8 re-throttle).
                for _ in range(n):
                    wps = mm_ps_pool.tile([P, 512], F32, name="wps", tag="ps")
                    nc.tensor.matmul(wps[:, 0:P], lhsT=ones, rhs=ones,
                                     start=True, stop=True)

            dummy_mms(WARMUP)

            # ---- input loads: x on sync, y on ACT queue (parallel DMA
            # engines), 512KB contiguous chunks.
            xt, yt = [], []
            for k in range(KC):
                xk = xin_pool.tile([P, N], BF16, name=f"xt{k}", tag=f"xt{k}")
                yk = yin_pool.tile([P, N], BF16, name=f"yt{k}", tag=f"yt{k}")
                # 3 parallel DMA paths balanced by measured bandwidth:
                # sync/ACT HWDGE ~155GB/s each, gpsimd SWDGE ~45GB/s.
                # gpsimd gets only the trailing halves of the k=3 pair
                # (512KB) so all three queues finish at ~the same time.
                ks = slice(k * P, (k + 1) * P)
                if k == KC - 1:
                    nc.sync.dma_start(out=xk[:, 0:1024], in_=xT[ks, 0:1024])
                    nc.scalar.dma_start(out=yk[:, 0:1024], in_=yT[ks, 0:1024])
                    nc.gpsimd.dma_start(out=xk[:, 1024:N], in_=xT[ks, 1024:N])
                    nc.gpsimd.dma_start(out=yk[:, 1024:N], in_=yT[ks, 1024:N])
                else:
                    # two 256KB sub-DMAs per chunk: more descriptors in
                    # flight engages more of the DMA engine pool
                    for h in (slice(0, 1024), slice(1024, N)):
                        nc.sync.dma_start(out=xk[:, h], in_=xT[ks, h])
                        nc.scalar.dma_start(out=yk[:, h], in_=yT[ks, h])
                xt.append(xk)
                yt.append(yk)

            # ---- per-chunk load-phase work, k-grouped ----------------
            ssqx = norm_pool.tile([P, NT], F32, name="ssqx")
            ry_ps = [
                ry_ps_pool.tile([P, 512], F32, name=f"ry_ps{c}", tag=f"ry{c}")
                for c in range(MC)
            ]
            # DVE queue order matters: squares for chunk k+1 are emitted
            # BEFORE the rx PSUM read of chunk k, so the square stream is
            # never blocked behind a PE round-trip (v7 lost ~5us/chunk to
            # that FIFO coupling).
            rxks = []
            for k in range(KC):
                xs = sq_pool.tile([P, N], BF16, name=f"xsq{k}", tag=f"xsq{k}")
                nc.vector.tensor_tensor(xs, xt[k], xt[k], mult)
                ys = sq_pool.tile([P, N], BF16, name=f"ysq{k}", tag=f"ysq{k}")
                nc.vector.tensor_tensor(ys, yt[k], yt[k], mult)
                if k > 0:
                    rk = rxks[k - 1]
                    if k == 1:
                        nc.vector.tensor_copy(ssqx, rk[:, 0:NT])
                    else:
                        nc.vector.tensor_tensor(ssqx, ssqx, rk[:, 0:NT],
                                                mybir.AluOpType.add)
                rxk = rx_ps_pool.tile([P, 512], F32, name=f"rx_ps{k}", tag="rx")
                rxks.append(rxk)
                for t in range(NT):
                    nc.tensor.matmul(
                        rxk[:, t:t + 1],
                        lhsT=xs[:, t * P:(t + 1) * P],
                        rhs=ones[:, 0:1],
                        start=True, stop=True,
                    )
                for c in range(MC):
                    nc.tensor.matmul(
                        ry_ps[c], lhsT=ones, rhs=ys[:, c * 512:(c + 1) * 512],
                        start=(k == 0), stop=(k == KC - 1),
                    )
                dummy_mms(FILLS[k])
            nc.vector.tensor_tensor(ssqx, ssqx, rxks[KC - 1][:, 0:NT],
                                    mybir.AluOpType.add)

            # ---- 1/sqrt via exp(-0.5*ln(s)); all Lns then all Exps ----
            lnx = norm_pool.tile([P, NT], F32, name="lnx")
            rx = norm_pool.tile([P, NT], F32, name="rx")
            lny = norm_pool.tile([P, N], F32, name="lny")
            ry = norm_pool.tile([P, N], BF16, name="ry")
            nc.scalar.activation(lnx, ssqx, Ln)
            for c in range(MC):
                cs = slice(c * 512, (c + 1) * 512)
                nc.scalar.activation(lny[:, cs], ry_ps[c], Ln)
            nc.scalar.activation(rx, lnx, Exp, scale=-0.5)
            for c in range(MC):
                cs = slice(c * 512, (c + 1) * 512)
                nc.scalar.activation(ry[:, cs], lny[:, cs], Exp, scale=-0.5)

            # ---- main matmuls + split epilogue ------------------------
            # k-inner accumulation, 3 rotating PSUM banks. Drain paths:
            #   defer (first 8 groups): DVE copy -> SBUF, scaled later
            #   ACT path (alternating):  tmp = Copy(ps*rx) bf16 on ACT,
            #                            ot = tmp * ry on DVE (327ns)
            #   DVE path (alternating):  ot = (ps*rx)*ry stt (750ns)
            ots, stage = [], []
            gidx = 0
            for t in range(NT):
                ts_ = slice(t * P, (t + 1) * P)
                ot = out_pool.tile([P, N], BF16, name="ot", tag="ot")
                ots.append(ot)
                for c in range(MC):
                    cs = slice(c * 512, (c + 1) * 512)
                    # after the load phase the ry/rx banks are dead: rotate
                    # main groups over all 8 PSUM banks so drain-semaphore
                    # latency stops pacing the MM stream (3-bank rotation
                    # measured ~250ns/MM vs the 213ns fill rate).
                    if gidx < DEFER:
                        ps = mm_ps_pool.tile([P, 512], F32, name="ps",
                                             tag="ps")
                    else:
                        sel = (gidx - DEFER) % 8
                        if sel < 3:
                            ps = mm_ps_pool.tile([P, 512], F32, name="ps",
                                                 tag="ps")
                        elif sel < 7:
                            ps = ry_ps_pool.tile([P, 512], F32, name="ps",
                                                 tag=f"ry{sel - 3}")
                        else:
                            ps = rx_ps_pool.tile([P, 512], F32, name="ps",
                                                 tag="rx")
                    for k in range(KC):
                        nc.tensor.matmul(
                            ps, lhsT=xt[k][:, ts_], rhs=yt[k][:, cs],
                            start=(k == 0), stop=(k == KC - 1),
                        )
                    if gidx < DEFER:
                        st = defer_pool.tile([P, 512], F32, name=f"st{gidx}",
                                             tag=f"st{gidx}")
                        nc.vector.tensor_copy(st, ps)
                        stage.append((t, c, st))
                    elif gidx % 2 == 0:
                        tmp = tmp_pool.tile([P, 512], BF16, name="tmp",
                                            tag="tmp")
                        nc.scalar.activation(tmp, ps, Copy,
                                             scale=rx[:, t:t + 1])
                        nc.vector.tensor_tensor(ot[:, cs], tmp, ry[:, cs],
                                                mult)
                    else:
                        nc.vector.scalar_tensor_tensor(
                            ot[:, cs], in0=ps, scalar=rx[:, t:t + 1],
                            in1=ry[:, cs], op0=mult, op1=mult,
                        )
                    gidx += 1
                    if gidx == DEFER:
                        # deferred epilogues: SBUF stt, waits rx/ry only.
                        # The deferred tiles' out-DMAs must be emitted
                        # AFTER these writes (emission order defines the
                        # dependency graph - v6.0 DMA'd unwritten SBUF).
                        for dt_, dc_, st_ in stage:
                            dcs = slice(dc_ * 512, (dc_ + 1) * 512)
                            nc.vector.scalar_tensor_tensor(
                                ots[dt_][:, dcs], in0=st_,
                                scalar=rx[:, dt_:dt_ + 1], in1=ry[:, dcs],
                                op0=mult, op1=mult,
                            )
                        for dt_ in range(DEFER // MC):
                            nc.sync.dma_start(
                                out=out[dt_ * P:(dt_ + 1) * P, :],
                                in_=ots[dt_])
                # contiguous 512KB row-block store on the sync HWDGE
                # queue (its input work ends at ~16us; ACT's queue now
                # carries the ACT-path epilogue compute instead).
                if t == NT - 1:
                    # last tile: per-chunk stores so the final DMA only
                    # covers 128KB after the last drain (tail shave)
                    for c in range(MC):
                        cs = slice(c * 512, (c + 1) * 512)
                        nc.sync.dma_start(out=out[ts_, cs], in_=ot[:, cs])
                elif t >= DEFER // MC:
                    nc.sync.dma_start(out=out[ts_, :], in_=ot)

    nc.compile()
    return nc


def _get_nc(variant: str = "v12") -> bass.Bass:
    if variant not in _CACHED:
        _CACHED[variant] = _build_nc(variant)
    return _CACHED[variant]


def _shard(x: np.ndarray, y: np.ndarray):
    """Host-side prep: cast to bf16 and transpose to [512, 2048]."""
    xq = np.asarray(x, dtype=np.float32).astype(ml_dtypes.bfloat16)
    yq = np.asarray(y, dtype=np.float32).astype(ml_dtypes.bfloat16)
    xTs = np.ascontiguousarray(np.transpose(xq, (0, 2, 1)))
    yTs = np.ascontiguousarray(np.transpose(yq, (0, 2, 1)))
    return [{"xT": xTs[b], "yT": yTs[b]} for b in range(B)]


def _run(x: np.ndarray, y: np.ndarray, variant: str = "v12",
         trace: bool = False):
    """Returns (out [8, 2048, 2048] f32, BassKernelResults)."""
    nc = _get_nc(variant)
    in_maps = _shard(x, y)
    res = run_bass_kernel_spmd(nc, in_maps, core_ids=list(range(B)), trace=trace)
    out = np.stack([res.results[b]["out"].astype(np.float32) for b in range(B)])
    return out, res


def kernel(x: np.ndarray, y: np.ndarray) -> np.ndarray:
    out, _ = _run(x, y)
    return out
